# revision 1
# baseline (speedup 1.0000x reference)
"""nn_MaxDistance Trainium2 kernel.

Problem: x, y: [8, 4096, 3] f32. Per batch b:
  d2[n,m] = ||x[b,n] - y[b,m]||^2
  h2[b] = max( max_n min_m d2, max_m min_n d2 )
  output = mean_b sqrt(h2[b])   (scalar f32)

Sharding: batch b -> NeuronCore b (8 cores, data parallel). Each core
computes its full 4096x4096 distance/min/max reduction; the final mean over
the 8 per-batch scalars is done on host (tiny all-reduce).

Device algorithm (per core):
  - The pairwise squared distance is computed on the TensorEngine as an
    augmented inner product: with a~ = (x0,x1,x2,||x||^2,1) and
    b~ = (-2y0,-2y1,-2y2,1,||y||^2),  d2[n,m] = a~_n . b~_m.
  - For full PE speed with near-fp32 accuracy, each f32 input value v is
    split on host into bf16 hi/lo parts (v = vh + vl); the K=5 augmented
    product becomes a K=15 bf16 matmul computing ah.bh + al.bh + ah.bl
    (the al.bl term, ~2^-18 relative, is dropped).
  - Each a-tile of 128 points is matmul'd against all 4096 b-points in
    512-column chunks into PSUM (f32), and the VectorEngine min-reduces
    PSUM groups into per-point minima; then max across points via
    reduce_max + a gpsimd partition_all_reduce, and the two directions are
    combined with an elementwise max. A single [1,1] f32 (squared
    Hausdorff) is DMA'd out per core.
"""

import numpy as np
import ml_dtypes

import concourse.bacc as bacc
import concourse.tile as tile
from concourse import mybir
from concourse import bass_utils
from concourse import bass_isa

P = 128
NPTS = 4096
D = 3
K = 15  # 5 augmented dims x 3 bf16 hi/lo product terms
BCH = 512  # matmul moving free dim (one PSUM bank of f32)
BIG = float(np.finfo(np.float32).max) / 4

BF16 = ml_dtypes.bfloat16

# variant: "reduce" = plain PSUM reduce_min (DVE only)
#          "mix16"  = ScalarE converts 6 of 8 PSUM banks per a-tile to fp16
#                     in SBUF; DVE min-combines those at 2x rate and
#                     reduces the remaining 2 banks directly in fp32
#          "ttr"    = tensor_tensor_reduce pairing (crashes TRN2 runtime —
#                     min-reduce uop missing; kept for reference)
VARIANT = "mix16"
MIX16_ACT_BANKS = 5  # of 8 PSUM banks routed through ScalarE
MIX16_NCP = 2       # ScalarE copies per a-tile
MIX16_GPSIMD_T1 = False  # run the first fp16 TT-min fold on GpSimd
GROUP = 2048  # b-columns consumed per DVE reduce op group

_NC_CACHE = {}


def _build_nc(variant=VARIANT, group=GROUP, npts=NPTS):
    if variant == "mix16":
        group = npts  # whole a-tile row in PSUM; bank-level deps pipeline it
    ntiles = npts // P
    ngroups = npts // group
    nmm = group // BCH
    half = group // 2
    psum_bufs = 1 if variant == "mix16" else 2

    nc = bacc.Bacc("TRN2", target_bir_lowering=False, debug=False)
    dt = mybir.dt

    ins = {}
    for name in ("xa", "yb", "ya", "xb"):
        ins[name] = nc.dram_tensor(name, [K, npts], dt.bfloat16,
                                   kind="ExternalInput").ap()
    out = nc.dram_tensor("h2", [1, 1], dt.float32, kind="ExternalOutput").ap()

    with tile.TileContext(nc) as tc:
        with (
            tc.tile_pool(name="singles", bufs=1) as singles,
            tc.tile_pool(name="psum", bufs=psum_bufs, space="PSUM") as psum_pool,
            tc.tile_pool(name="cp", bufs=3) as cp_pool,
            tc.tile_pool(name="trash", bufs=1) as trash_pool,
            tc.tile_pool(name="accs", bufs=1) as accs_pool,
            tc.tile_pool(name="fin", bufs=1) as fin_pool,
        ):
            ab = {}
            for name in ("xa", "yb", "ya", "xb"):
                t = singles.tile([K, npts], dt.bfloat16, tag=name,
                                 name=f"pts_{name}")
                nc.sync.dma_start(out=t, in_=ins[name])
                ab[name] = t

            dirs = ((ab["xa"], ab["yb"]), (ab["ya"], ab["xb"]))
            accs = [accs_pool.tile([P, ntiles, ngroups], dt.float32,
                                   name=f"acc{d}") for d in range(2)]
            if variant == "ttr":
                dummy = trash_pool.tile([P, 1], dt.float32, name="dummy")

            for d, (A, B) in enumerate(dirs):
                for t in range(ntiles):
                    lhsT = A[:, t * P:(t + 1) * P]
                    for g in range(ngroups):
                        pp = psum_pool.tile([P, group], dt.float32, tag="pp")
                        for j in range(nmm):
                            nc.tensor.matmul(
                                out=pp[:, j * BCH:(j + 1) * BCH],
                                lhsT=lhsT,
                                rhs=B[:, g * group + j * BCH:
                                      g * group + (j + 1) * BCH],
                                start=True, stop=True,
                            )
                        if variant == "mix16":
                            # First MIX16_ACT_BANKS banks -> fp16 SBUF via
                            # ScalarE (few wide copies amortize the ACT
                            # per-op init, which dominates at 1024 wide);
                            # remaining banks reduced directly from PSUM in
                            # fp32 on the DVE.
                            acols = MIX16_ACT_BANKS * BCH
                            ncp = MIX16_NCP
                            w = acols // ncp
                            cps = []
                            for ci in range(ncp):
                                cp = cp_pool.tile([P, w], dt.float16,
                                                  tag=f"cp{ci}")
                                nc.scalar.copy(
                                    out=cp, in_=pp[:, ci * w:(ci + 1) * w])
                                cps.append(cp)
                            r67 = trash_pool.tile([P, 1], dt.float32,
                                                  tag="r67", bufs=2)
                            nc.vector.tensor_reduce(
                                out=r67, in_=pp[:, acols:group],
                                axis=mybir.AxisListType.X,
                                op=mybir.AluOpType.min)
                            # fold the fp16 copies with 2x-rate TT-mins,
                            # halving until narrow enough to reduce
                            cur = cps[0]
                            cw = w
                            ti = 0
                            for ci in range(1, ncp):
                                nxt = cp_pool.tile([P, cw], dt.float16,
                                                   tag=f"z{ti}")
                                nc.vector.tensor_tensor(
                                    out=nxt, in0=cur, in1=cps[ci],
                                    op=mybir.AluOpType.min)
                                cur = nxt
                                ti += 1
                            while cw > 512:
                                cw //= 2
                                nxt = cp_pool.tile([P, cw], dt.float16,
                                                   tag=f"z{ti}")
                                nc.vector.tensor_tensor(
                                    out=nxt, in0=cur[:, 0:cw],
                                    in1=cur[:, cw:2 * cw],
                                    op=mybir.AluOpType.min)
                                cur = nxt
                                ti += 1
                            t4 = trash_pool.tile([P, 1], dt.float16,
                                                 tag="t4", bufs=2)
                            nc.vector.tensor_reduce(
                                out=t4, in_=cur, axis=mybir.AxisListType.X,
                                op=mybir.AluOpType.min)
                            nc.vector.tensor_tensor(
                                out=accs[d][:, t, g:g + 1], in0=r67, in1=t4,
                                op=mybir.AluOpType.min)
                        elif variant == "ttr":
                            cp = cp_pool.tile([P, half], dt.float32, tag="cp")
                            nc.scalar.copy(out=cp, in_=pp[:, half:group])
                            nc.vector.tensor_tensor_reduce(
                                out=dummy.broadcast_to((P, half)),
                                in0=pp[:, 0:half],
                                in1=cp,
                                scale=1.0,
                                scalar=BIG,
                                op0=mybir.AluOpType.min,
                                op1=mybir.AluOpType.min,
                                accum_out=accs[d][:, t, g:g + 1],
                            )
                        else:
                            nc.vector.tensor_reduce(
                                out=accs[d][:, t, g:g + 1], in_=pp,
                                axis=mybir.AxisListType.X,
                                op=mybir.AluOpType.min)

            hmaxes = fin_pool.tile([P, 2], dt.float32, name="hmaxes")
            for d in range(2):
                amin = fin_pool.tile([P, ntiles], dt.float32, name=f"amin{d}")
                nc.vector.tensor_reduce(
                    out=amin, in_=accs[d], axis=mybir.AxisListType.X,
                    op=mybir.AluOpType.min)
                nc.vector.tensor_reduce(
                    out=hmaxes[:, d:d + 1], in_=amin,
                    axis=mybir.AxisListType.X, op=mybir.AluOpType.max)
            hb = fin_pool.tile([P, 1], dt.float32, name="hb")
            nc.vector.tensor_tensor(
                out=hb, in0=hmaxes[:, 0:1], in1=hmaxes[:, 1:2],
                op=mybir.AluOpType.max)
            hred = fin_pool.tile([P, 1], dt.float32, name="hred")
            nc.gpsimd.partition_all_reduce(
                out_ap=hred, in_ap=hb, channels=P,
                reduce_op=bass_isa.ReduceOp.max)
            nc.sync.dma_start(out=out, in_=hred[0:1, 0:1])

    nc.compile()
    return nc


def get_nc(**kw):
    key = tuple(sorted(kw.items()))
    if key not in _NC_CACHE:
        _NC_CACHE[key] = _build_nc(**kw)
    return _NC_CACHE[key]


def _split_rows(rows_f32):
    """rows_f32: [5, n] f32 -> hi/lo interleaved [15, n] bf16 pair pattern.

    For a-side array SA and b-side array SB the matmul computes
    sum_k SA[k].SB[k]; rows are laid out so that per augmented dim i:
      a rows: (ah, al, ah)   b rows: (bh, bh, bl)
    giving ah.bh + al.bh + ah.bl per dim."""
    hi = rows_f32.astype(BF16)
    lo = (rows_f32 - hi.astype(np.float32)).astype(BF16)
    return hi, lo


def _make_core_inputs(xb_, yb_):
    """xb_, yb_: [4096, 3] f32 for one batch -> input dict for one core."""
    def aug_a(p):
        n = (p * p).sum(axis=1, dtype=np.float32)
        return np.stack([p[:, 0], p[:, 1], p[:, 2], n,
                         np.ones_like(n)], 0).astype(np.float32)

    def aug_b(p):
        n = (p * p).sum(axis=1, dtype=np.float32)
        return np.stack([-2 * p[:, 0], -2 * p[:, 1], -2 * p[:, 2],
                         np.ones_like(n), n], 0).astype(np.float32)

    def a_side(rows):
        hi, lo = _split_rows(rows)
        outr = np.empty((K, rows.shape[1]), BF16)
        outr[0::3] = hi
        outr[1::3] = lo
        outr[2::3] = hi
        return outr

    def b_side(rows):
        hi, lo = _split_rows(rows)
        outr = np.empty((K, rows.shape[1]), BF16)
        outr[0::3] = hi
        outr[1::3] = hi
        outr[2::3] = lo
        return outr

    return {
        "xa": np.ascontiguousarray(a_side(aug_a(xb_))),
        "yb": np.ascontiguousarray(b_side(aug_b(yb_))),
        "ya": np.ascontiguousarray(a_side(aug_a(yb_))),
        "xb": np.ascontiguousarray(b_side(aug_b(xb_))),
    }


def kernel(x, y):
    x = np.asarray(x, dtype=np.float32)
    y = np.asarray(y, dtype=np.float32)
    nbatch = x.shape[0]
    nc = get_nc()
    in_maps = [_make_core_inputs(x[b], y[b]) for b in range(nbatch)]
    res = bass_utils.run_bass_kernel_spmd(
        nc, in_maps, core_ids=list(range(nbatch)))
    h2 = np.array([res.results[b]["h2"][0, 0] for b in range(nbatch)],
                  dtype=np.float32)
    return np.float32(np.sqrt(np.maximum(h2, 0.0)).mean())



# revision 11
# speedup vs baseline: 1.4693x; 1.4693x over previous
"""nn_MaxDistance Trainium2 kernel (single-pass dual-reduction).

Problem: x, y: [8, 4096, 3] f32. Per batch b:
  d2[n,m] = ||x[b,n] - y[b,m]||^2
  h2[b] = max( max_n min_m d2, max_m min_n d2 )
  output = mean_b sqrt(h2[b])   (scalar f32)

Sharding: batch b -> NeuronCore b (8 cores, data parallel). Final mean of the
8 per-batch scalars on host.

Device algorithm (per core) — the d2 matrix is computed ONCE and reduced
along BOTH axes (the baseline ran two transposed passes, doubling PE work):

  - d2 via augmented inner product on the PE: a~=(x0,x1,x2,||x||^2,1),
    b~=(-2y0,-2y1,-2y2,1,||y||^2); bf16 hi/lo split -> K=15 bf16 rows.
  - Iteration = (x-tile pair, y-quarter): PSUM [128, 2, 1024] f32 (4 banks),
    2 buffers ping-pong. 4 matmuls fill it (tiles A,B x 2 512-col chunks).
  - Every PSUM value feeds two reductions (row-min over y per x-point and
    col-min over x per y-point). HW allows at most ONE PSUM operand per
    vector op, so iterations are routed across engines to balance load:
    "d": row = tensor_reduce straight off PSUM (1x), col = two
         TT-min(acc_fp16_sbuf, psum_tile) mixed ops (1x). No ACT.
    "a": ACT converts the 2048 f32 to fp16 SBUF once; DVE does row L1 +
         col pair/acc as fp16 SBUF TTs at 2x.
    "p": like "a" but the col pair/acc run on GpSimd (Pool).
  - Row halves land in 512-wide fp16 rowbuf slots; per tile-pair a TT-min
    halving tree + tensor_reduce collapses them to [2] rowmins ("d" rows
    bypass the tree via their direct TR). Some pairs' trees run on Pool.
  - Col acc[128, 4096] fp16 = min over all x with x%128 = partition;
    PE-transposes 32 [128,128] blocks into PSUM, ACT converts them back to
    SBUF, and a TT/TR tree finishes min-over-partitions then max-over-y.
  - Final: max(row dir, col dir) -> gpsimd partition_all_reduce(max) ->
    [1,1] f32 squared-Hausdorff out; host does sqrt + mean.
"""

import numpy as np
import ml_dtypes

import concourse.bacc as bacc
import concourse.tile as tile
from concourse import mybir
from concourse import bass_utils
from concourse import bass_isa

P = 128
NPTS = 4096
K = 15          # 5 augmented dims x 3 bf16 hi/lo product terms
NTILES = NPTS // P          # 32 x-tiles
NPAIRS = NTILES // 2        # 16 tile pairs
QCOLS = 1024                # y-columns per iteration
NQ = NPTS // QCOLS          # 4 quarters
NIT = NPAIRS * NQ           # 64 iterations

BF16 = ml_dtypes.bfloat16

# Routing knobs: counts of DVE-direct and ACT+DVE iterations (rest are
# ACT+Pool); which tile-pairs run their row-fold tree on Pool.
N_D = 0
N_A = 64
TREE_POOL = frozenset()  # Pool TT fails neuronxcc codegen; keep empty

_NC_CACHE = {}


def _route_list():
    """Spread d/a/p routes evenly across the 64 iterations."""
    n_p = NIT - N_D - N_A
    routes = []
    # largest-remainder style round-robin by cumulative quota
    cd = ca = cp = 0.0
    for it in range(NIT):
        want_d = (it + 1) * N_D / NIT
        want_a = (it + 1) * N_A / NIT
        want_p = (it + 1) * n_p / NIT
        # pick the route most behind its quota
        gaps = [(want_d - cd, "d"), (want_a - ca, "a"), (want_p - cp, "p")]
        gaps.sort(reverse=True)
        r = gaps[0][1]
        routes.append(r)
        if r == "d":
            cd += 1
        elif r == "a":
            ca += 1
        else:
            cp += 1
    return routes


def _build_nc():
    nc = bacc.Bacc("TRN2", target_bir_lowering=False, debug=False)
    dt = mybir.dt
    alu = mybir.AluOpType
    ax = mybir.AxisListType

    ins = {}
    for name in ("xa", "yb"):
        ins[name] = nc.dram_tensor(name, [K, NPTS], dt.bfloat16,
                                   kind="ExternalInput").ap()
    ins["ident"] = nc.dram_tensor("ident", [P, P], dt.float16,
                                  kind="ExternalInput").ap()
    out = nc.dram_tensor("h2", [1, 1], dt.float32, kind="ExternalOutput").ap()

    routes = _route_list()

    with tile.TileContext(nc) as tc:
        with (
            tc.tile_pool(name="singles", bufs=1) as singles,
            tc.tile_pool(name="psum", bufs=2, space="PSUM") as psum_pool,
            tc.tile_pool(name="cv", bufs=3) as cv_pool,
            tc.tile_pool(name="mbuf", bufs=3) as m_pool,
            tc.tile_pool(name="rowb", bufs=2) as row_pool,
            tc.tile_pool(name="tree", bufs=2) as tree_pool,
            tc.tile_pool(name="fin", bufs=1) as fin_pool,
        ):
            ab = {}
            for name in ("xa", "yb"):
                t = singles.tile([K, NPTS], dt.bfloat16, tag=name,
                                 name=f"pts_{name}")
                nc.sync.dma_start(out=t, in_=ins[name])
                ab[name] = t
            A_ = ab["xa"]
            B_ = ab["yb"]

            # col-path accumulator: [128, q, 1024] fp16, init +inf-ish
            acc = fin_pool.tile([P, NQ, QCOLS], dt.float16, name="acc")
            rowmins = fin_pool.tile([P, NTILES], dt.float16, name="rowmins")

            it = 0
            for tp in range(NPAIRS):
                pair_routes = routes[it:it + NQ]
                k_ap = sum(1 for r in pair_routes if r != "d")
                k_d = NQ - k_ap
                rb = row_pool.tile([P, 2, 2048], dt.float16, tag="rb",
                                   name="rb")
                if k_d:
                    rowd = row_pool.tile([P, 2, NQ], dt.float16, tag="rowd",
                                         name="rowd")
                slot_i = 0
                d_i = 0
                for q in range(NQ):
                    route = routes[it]
                    it += 1
                    pp = psum_pool.tile([P, 2, QCOLS], dt.float32, tag="pp",
                                        name="pp")
                    for half in range(2):
                        t_idx = 2 * tp + half
                        lhsT = A_[:, t_idx * P:(t_idx + 1) * P]
                        for j in range(2):
                            c0 = q * QCOLS + j * 512
                            nc.tensor.matmul(
                                out=pp[:, half, j * 512:(j + 1) * 512],
                                lhsT=lhsT,
                                rhs=B_[:, c0:c0 + 512],
                                start=True, stop=True,
                            )
                    if route == "d":
                        # row: full min over this iteration's cols, one TR
                        nc.vector.tensor_reduce(
                            out=rowd[:, :, d_i:d_i + 1], in_=pp,
                            axis=ax.X, op=alu.min)
                        d_i += 1
                        # col: fold both tiles into acc (mixed fp32/fp16, 1x)
                        nc.vector.tensor_tensor(
                            out=acc[:, q, :], in0=acc[:, q, :],
                            in1=pp[:, 0, :], op=alu.min)
                        nc.vector.tensor_tensor(
                            out=acc[:, q, :], in0=acc[:, q, :],
                            in1=pp[:, 1, :], op=alu.min)
                    else:
                        cv = cv_pool.tile([P, 2, QCOLS], dt.float16, tag="cv",
                                          name="cv")
                        nc.scalar.copy(out=cv, in_=pp)
                        rslot = rb[:, :, 512 * slot_i:512 * (slot_i + 1)]
                        slot_i += 1
                        nc.vector.tensor_tensor(
                            out=rslot, in0=cv[:, :, 0:512],
                            in1=cv[:, :, 512:1024], op=alu.min)
                        if tp == 0:
                            # first pair seeds acc directly: no memset needed
                            nc.vector.tensor_tensor(
                                out=acc[:, q, :], in0=cv[:, 0, :],
                                in1=cv[:, 1, :], op=alu.min)
                        else:
                            M = m_pool.tile([P, QCOLS], dt.float16, tag="m",
                                            name="m")
                            nc.vector.tensor_tensor(
                                out=M, in0=cv[:, 0, :], in1=cv[:, 1, :],
                                op=alu.min)
                            nc.vector.tensor_tensor(
                                out=acc[:, q, :], in0=acc[:, q, :], in1=M,
                                op=alu.min)

                # row fold for this tile pair -> rowmins[:, 2tp:2tp+2].
                # Tree over the k_ap fp16 slots; "d" rows came pre-reduced
                # in rowd.
                rmout = rowmins[:, 2 * tp:2 * tp + 2]
                teng = nc.vector
                tree_out = None
                if k_ap:
                    # pairwise-fold slots to one 512-wide piece
                    pieces = [rb[:, :, 512 * i:512 * (i + 1)]
                              for i in range(k_ap)]
                    ti = 0
                    while len(pieces) > 1:
                        fold = []
                        for i in range(0, len(pieces) - 1, 2):
                            nxt = tree_pool.tile([P, 2, 512], dt.float16,
                                                 tag=f"f{ti}", name=f"f{ti}")
                            teng.tensor_tensor(
                                out=nxt, in0=pieces[i], in1=pieces[i + 1],
                                op=alu.min)
                            fold.append(nxt)
                            ti += 1
                        if len(pieces) % 2:
                            fold.append(pieces[-1])
                        pieces = fold
                    cur = pieces[0]
                    w = 256
                    while w >= 32:
                        nxt = tree_pool.tile([P, 2, w], dt.float16,
                                             tag=f"t{w}", name=f"t{w}")
                        teng.tensor_tensor(
                            out=nxt, in0=cur[:, :, 0:w], in1=cur[:, :, w:2 * w],
                            op=alu.min)
                        cur = nxt
                        w //= 2
                    if k_d == 0:
                        nc.vector.tensor_reduce(
                            out=rmout, in_=cur, axis=ax.X, op=alu.min)
                    else:
                        tree_out = tree_pool.tile([P, 2], dt.float16,
                                                  tag="tro", name="tro")
                        nc.vector.tensor_reduce(
                            out=tree_out, in_=cur, axis=ax.X, op=alu.min)
                if k_d:
                    if k_ap == 0:
                        nc.vector.tensor_reduce(
                            out=rmout, in_=rowd[:, :, 0:k_d],
                            axis=ax.X, op=alu.min)
                    else:
                        dro = tree_pool.tile([P, 2], dt.float16,
                                             tag="dro", name="dro")
                        nc.vector.tensor_reduce(
                            out=dro, in_=rowd[:, :, 0:k_d],
                            axis=ax.X, op=alu.min)
                        nc.vector.tensor_tensor(
                            out=rmout, in0=tree_out, in1=dro, op=alu.min)

            # ---- endgame ----
            # row direction: max over tiles then over partitions
            hrow = fin_pool.tile([P, 1], dt.float16, name="hrow")
            nc.vector.tensor_reduce(out=hrow, in_=rowmins, axis=ax.X,
                                    op=alu.max)

            # col direction: transpose acc blocks, min over partitions,
            # max over y
            ident = fin_pool.tile([P, P], dt.float16, name="ident")
            nc.sync.dma_start(out=ident, in_=ins["ident"])
            es = fin_pool.tile([P, 32, P], dt.float16, name="es")
            for tph in range(2):
                pt = psum_pool.tile([P, 16, P], dt.float16, tag="pp",
                                    name="pt")
                for blk in range(16):
                    c = tph * 16 + blk
                    nc.tensor.transpose(
                        out=pt[:, blk, :],
                        in_=acc[:, c // 8, (c % 8) * P:(c % 8 + 1) * P],
                        identity=ident)
                nc.scalar.copy(out=es[:, tph * 16:(tph + 1) * 16, :],
                               in_=pt)
            cur = es
            w = 64
            ti = 0
            while w >= 4:
                nxt = fin_pool.tile([P, 32, w], dt.float16, name=f"e{ti + 2}")
                nc.vector.tensor_tensor(
                    out=nxt, in0=cur[:, :, 0:w], in1=cur[:, :, w:2 * w],
                    op=alu.min)
                cur = nxt
                w //= 2
                ti += 1
            colmin = fin_pool.tile([P, 32], dt.float16, name="colmin")
            nc.vector.tensor_reduce(out=colmin, in_=cur, axis=ax.X,
                                    op=alu.min)
            hcol = fin_pool.tile([P, 1], dt.float16, name="hcol")
            nc.vector.tensor_reduce(out=hcol, in_=colmin, axis=ax.X,
                                    op=alu.max)

            hb = fin_pool.tile([P, 1], dt.float32, name="hb")
            nc.vector.tensor_tensor(out=hb, in0=hrow, in1=hcol, op=alu.max)
            hred = fin_pool.tile([P, 1], dt.float32, name="hred")
            nc.gpsimd.partition_all_reduce(
                out_ap=hred, in_ap=hb, channels=P,
                reduce_op=bass_isa.ReduceOp.max)
            nc.sync.dma_start(out=out, in_=hred[0:1, 0:1])

    nc.compile()
    return nc


def get_nc():
    if "nc" not in _NC_CACHE:
        _NC_CACHE["nc"] = _build_nc()
    return _NC_CACHE["nc"]


def _split_rows(rows_f32):
    """rows_f32: [5, n] f32 -> bf16 hi/lo parts."""
    hi = rows_f32.astype(BF16)
    lo = (rows_f32 - hi.astype(np.float32)).astype(BF16)
    return hi, lo


def _make_core_inputs(xb_, yb_):
    """xb_, yb_: [4096, 3] f32 for one batch -> input dict for one core.

    Per augmented dim i the K=15 rows pair as a:(ah, al, ah) vs
    b:(bh, bh, bl), giving ah.bh + al.bh + ah.bl (al.bl ~2^-18 dropped)."""
    def aug_a(p):
        n = (p * p).sum(axis=1, dtype=np.float32)
        return np.stack([p[:, 0], p[:, 1], p[:, 2], n,
                         np.ones_like(n)], 0).astype(np.float32)

    def aug_b(p):
        n = (p * p).sum(axis=1, dtype=np.float32)
        return np.stack([-2 * p[:, 0], -2 * p[:, 1], -2 * p[:, 2],
                         np.ones_like(n), n], 0).astype(np.float32)

    def a_side(rows):
        hi, lo = _split_rows(rows)
        outr = np.empty((K, rows.shape[1]), BF16)
        outr[0::3] = hi
        outr[1::3] = lo
        outr[2::3] = hi
        return outr

    def b_side(rows):
        hi, lo = _split_rows(rows)
        outr = np.empty((K, rows.shape[1]), BF16)
        outr[0::3] = hi
        outr[1::3] = hi
        outr[2::3] = lo
        return outr

    return {
        "xa": np.ascontiguousarray(a_side(aug_a(xb_))),
        "yb": np.ascontiguousarray(b_side(aug_b(yb_))),
        "ident": np.eye(P, dtype=np.float16),
    }


def kernel(x, y):
    x = np.asarray(x, dtype=np.float32)
    y = np.asarray(y, dtype=np.float32)
    nbatch = x.shape[0]
    nc = get_nc()
    in_maps = [_make_core_inputs(x[b], y[b]) for b in range(nbatch)]
    res = bass_utils.run_bass_kernel_spmd(
        nc, in_maps, core_ids=list(range(nbatch)))
    h2 = np.array([res.results[b]["h2"][0, 0] for b in range(nbatch)],
                  dtype=np.float32)
    return np.float32(np.sqrt(np.maximum(h2, 0.0)).mean())
